# revision 6
# baseline (speedup 1.0000x reference)
"""BertAttention (cross-attention variant) Trainium2 Bass kernel.

Strategy: data-parallel over batch (16 batches -> 8 cores x 2 batches).
Each core independently computes, for its 2 batches:
  Q^T = Wq^T X^T, K^T = Wk^T C^T (transposed layouts, head-sliced),
  V (natural layout, with an appended ones-column per head for the
  softmax denominator), S^T = K Q^T per head (row-packed pairs of
  heads on the PE), P = exp(S/8) (no max-subtraction needed: scores
  are O(1) by construction), O[q, 65] = sum_k P^T[k,q]^T V_aug[k, 65];
  the last column gives the softmax denominator; normalize with a
  per-partition reciprocal on the vector engine.

All matmul operands are bf16 (fp32 PSUM accumulation). Activations are
transposed via a DRAM bf16 bounce + hardware DMA-transpose (X-bar).
"""

import os
import sys

import numpy as np

for _p in ("/opt/trn_rl_repo", "/root/.axon_site/_ro/trn_rl_repo"):
    if os.path.isdir(_p) and _p not in sys.path:
        sys.path.insert(0, _p)

import concourse.bass as bass  # noqa: E402
import concourse.tile as tile  # noqa: E402
from concourse import bacc, mybir  # noqa: E402
from concourse.bass_utils import run_bass_kernel_spmd  # noqa: E402

# Problem constants (hardcoded per spec)
B, S, D, H, HD = 16, 512, 768, 12, 64
NCORES = 8
BL = B // NCORES  # batches per core = 2
DT = D // 128     # 6 d-tiles
KT = S // 128     # 4 k-token tiles
QT = S // 128     # 4 q-token tiles
HP = H // 2       # 6 head pairs

f32 = mybir.dt.float32
bf16 = mybir.dt.bfloat16
AF = mybir.ActivationFunctionType

_CACHE = {}


def _emit(tc, hs, ct, w_aps, b_aps, out):
    nc = tc.nc
    from contextlib import ExitStack

    with ExitStack() as ctx:
        wpool = ctx.enter_context(tc.tile_pool(name="wpool", bufs=1))
        xtp = ctx.enter_context(tc.tile_pool(name="xtp", bufs=4))
        qkp = ctx.enter_context(tc.tile_pool(name="qkp", bufs=4))
        vap = ctx.enter_context(tc.tile_pool(name="vap", bufs=2))
        exps_p = ctx.enter_context(tc.tile_pool(name="exps_p", bufs=10))
        orow_p = ctx.enter_context(tc.tile_pool(name="orow_p", bufs=8))
        small_p = ctx.enter_context(tc.tile_pool(name="small_p", bufs=16))
        dram_p = ctx.enter_context(tc.tile_pool(name="dram_p", bufs=2, space="DRAM"))
        psum_p = ctx.enter_context(tc.tile_pool(name="psum_p", bufs=3, space="PSUM"))
        pv_p = ctx.enter_context(tc.tile_pool(name="pv_p", bufs=2, space="PSUM"))

        # ---- weights / biases (resident, cast to bf16 on load) ----
        w_sb = {}
        for name in ("q", "k", "v"):
            wt = wpool.tile([128, DT, D], bf16, name=f"w_{name}")
            nc.gpsimd.dma_start(
                out=wt, in_=w_aps[name].rearrange("(a p) d -> p a d", p=128)
            )
            w_sb[name] = wt
        bq_sb = wpool.tile([128, DT], f32, name="bq_sb")
        nc.sync.dma_start(out=bq_sb, in_=b_aps["q"].rearrange("(a p) -> p a", p=128))
        bk_sb = wpool.tile([128, DT], f32, name="bk_sb")
        nc.sync.dma_start(out=bk_sb, in_=b_aps["k"].rearrange("(a p) -> p a", p=128))
        bv_sb = wpool.tile([128, H, HD], f32, name="bv_sb")
        bv = b_aps["v"]
        bv_bcast = bass.AP(tensor=bv.tensor, offset=bv.offset, ap=[[0, 128], [1, D]])
        nc.gpsimd.dma_start(out=bv_sb, in_=bv_bcast)

        # ---- per-batch input staging: cast to bf16 in DRAM, then
        #      hardware DMA-transpose into SBUF ----
        def stage_in(b):
            hsb = dram_p.tile([DT, S, 128], bf16, name="hs_bf")
            ctb = dram_p.tile([DT, S, 128], bf16, name="ct_bf")
            nc.gpsimd.dma_start(out=hsb, in_=hs[b].rearrange("t (a d) -> a t d", d=128))
            nc.gpsimd.dma_start(out=ctb, in_=ct[b].rearrange("t (a d) -> a t d", d=128))
            # The DMA_DIRECT2D_XPOSE descriptor supports very few sync-wait
            # slots; absorb the cast-DMA completion waits into plain DMAs on
            # the same (SP) queue so the transposes themselves need none.
            dummy = small_p.tile([1, 8], bf16, name="dummy")
            nc.sync.dma_start(out=dummy[0:1, 0:4], in_=hsb[0, 0:1, 0:4])
            nc.sync.dma_start(out=dummy[0:1, 4:8], in_=ctb[0, 0:1, 0:4])
            xt = xtp.tile([128, DT, S], bf16, name="xt")
            ctt = xtp.tile([128, DT, S], bf16, name="ctt")
            for dt_ in range(DT):
                nc.sync.dma_start(out=xt[:, dt_, :], in_=hsb[dt_], transpose=True)
                nc.sync.dma_start(out=ctt[:, dt_, :], in_=ctb[dt_], transpose=True)
            return xt, ctt

        # ---- projection chunk closures for one batch ----
        def proj_chunks(xt, ctt, store):
            qt_t = qkp.tile([128, DT, S], bf16, name="qt_t")
            kt_t = qkp.tile([128, DT, S], bf16, name="kt_t")
            va_t = vap.tile([128, KT, H, HD + 1], bf16, name="va_t")
            store["qt"], store["kt"], store["va"] = qt_t, kt_t, va_t
            chunks = []
            for wname, src, dstT, bias_sb in (
                ("q", xt, qt_t, bq_sb),
                ("k", ctt, kt_t, bk_sb),
            ):
                for m in range(DT):
                    def f(wname=wname, src=src, dstT=dstT, bias_sb=bias_sb, m=m):
                        ps = psum_p.tile([128, 1024], f32, tag="big", name="ps_big")
                        for k in range(DT):
                            nc.tensor.matmul(
                                ps[:, 0:S],
                                lhsT=w_sb[wname][:, k, m * 128:(m + 1) * 128],
                                rhs=src[:, k, :],
                                start=(k == 0),
                                stop=(k == DT - 1),
                            )
                        nc.vector.tensor_scalar_add(
                            out=dstT[:, m, :],
                            in0=ps[:, 0:S],
                            scalar1=bias_sb[:, m:m + 1],
                        )
                    chunks.append(f)
            for m in range(KT):
                def f(m=m):
                    ps = psum_p.tile([128, 1024], f32, tag="big", name="ps_big")
                    for lo, hi in ((0, 512), (512, 768)):
                        for k in range(DT):
                            nc.tensor.matmul(
                                ps[:, lo:hi],
                                lhsT=ctt[:, k, m * 128:(m + 1) * 128],
                                rhs=w_sb["v"][:, k, lo:hi],
                                start=(k == 0),
                                stop=(k == DT - 1),
                            )
                    ps_h = ps[:, 0:D].rearrange("p (h x) -> p h x", x=HD)
                    nc.vector.tensor_add(
                        out=va_t[:, m, :, 0:HD], in0=ps_h, in1=bv_sb
                    )
                    nc.vector.memset(va_t[:, m, :, HD:HD + 1], 1.0)
                chunks.append(f)
            return chunks

        # ---- one attention head-pair for one batch ----
        def attn_pair(store, hp, orows):
            qt_t, kt_t, va_t = store["qt"], store["kt"], store["va"]
            exps_tiles = []
            for kt in range(KT):
                st = psum_p.tile([128, 2, S], f32, tag="big", name="st")
                for pr in (0, 1):
                    nc.tensor.matmul(
                        st[:, pr, :],
                        lhsT=kt_t[pr * 64:(pr + 1) * 64, hp, kt * 128:(kt + 1) * 128],
                        rhs=qt_t[pr * 64:(pr + 1) * 64, hp, :],
                        start=True,
                        stop=True,
                        tile_position=(pr * 64, 0),
                    )
                ex = exps_p.tile([128, 2, S], bf16, name="ex")
                nc.scalar.activation(out=ex, in_=st, func=AF.Exp, scale=0.125)
                exps_tiles.append(ex)
            for pr in (0, 1):
                h = 2 * hp + pr
                pv = pv_p.tile([128, QT, HD + 1], f32, tag="pv", name="pv")
                for q in range(QT):
                    for kt in range(KT):
                        nc.tensor.matmul(
                            pv[:, q, :],
                            lhsT=exps_tiles[kt][:, pr, q * 128:(q + 1) * 128],
                            rhs=va_t[:, kt, h, :],
                            start=(kt == 0),
                            stop=(kt == KT - 1),
                        )
                for q in range(QT):
                    rc = small_p.tile([128, 1], f32, name="rc")
                    nc.vector.reciprocal(rc, pv[:, q, HD:HD + 1])
                    nc.vector.tensor_scalar_mul(
                        out=orows[q][:, h, :],
                        in0=pv[:, q, 0:HD],
                        scalar1=rc,
                    )

        # ---- schedule: software-pipeline batch 1's projections into
        #      batch 0's attention to keep the PE dense ----
        stores = [{}, {}]
        xt0, ct0 = stage_in(0)
        ch0 = proj_chunks(xt0, ct0, stores[0])
        for f in ch0:
            f()
        xt1, ct1 = stage_in(1)
        ch1 = proj_chunks(xt1, ct1, stores[1])

        orows0 = [orow_p.tile([128, H, HD], f32, name="orow") for _ in range(QT)]
        for hp in range(HP):
            attn_pair(stores[0], hp, orows0)
            for _ in range(3):
                if ch1:
                    ch1.pop(0)()
        while ch1:
            ch1.pop(0)()
        for q in range(QT):
            nc.sync.dma_start(out=out[0, q * 128:(q + 1) * 128, :], in_=orows0[q])

        orows1 = [orow_p.tile([128, H, HD], f32, name="orow") for _ in range(QT)]
        for hp in range(HP):
            attn_pair(stores[1], hp, orows1)
        for q in range(QT):
            nc.sync.dma_start(out=out[1, q * 128:(q + 1) * 128, :], in_=orows1[q])


def build_program():
    if "nc" in _CACHE:
        return _CACHE["nc"]
    nc = bacc.Bacc("TRN2", target_bir_lowering=False, debug=False)
    hs = nc.dram_tensor("hs", [BL, S, D], f32, kind="ExternalInput").ap()
    ct = nc.dram_tensor("ct", [BL, S, D], f32, kind="ExternalInput").ap()
    w_aps = {
        n: nc.dram_tensor(f"w{n}", [D, D], f32, kind="ExternalInput").ap()
        for n in ("q", "k", "v")
    }
    b_aps = {
        n: nc.dram_tensor(f"b{n}", [D], f32, kind="ExternalInput").ap()
        for n in ("q", "k", "v")
    }
    out = nc.dram_tensor("out", [BL, S, D], f32, kind="ExternalOutput").ap()
    with tile.TileContext(nc) as tc:
        _emit(tc, hs, ct, w_aps, b_aps, out)
    nc.compile()
    _CACHE["nc"] = nc
    return nc


def make_in_maps(hidden_states, context, Wq, bq, Wk, bk, Wv, bv):
    hidden_states = np.ascontiguousarray(np.asarray(hidden_states, np.float32))
    context = np.ascontiguousarray(np.asarray(context, np.float32))
    common = {
        "wq": np.ascontiguousarray(np.asarray(Wq, np.float32)),
        "wk": np.ascontiguousarray(np.asarray(Wk, np.float32)),
        "wv": np.ascontiguousarray(np.asarray(Wv, np.float32)),
        "bq": np.ascontiguousarray(np.asarray(bq, np.float32)),
        "bk": np.ascontiguousarray(np.asarray(bk, np.float32)),
        "bv": np.ascontiguousarray(np.asarray(bv, np.float32)),
    }
    in_maps = []
    for c in range(NCORES):
        m = dict(common)
        m["hs"] = np.ascontiguousarray(hidden_states[c * BL:(c + 1) * BL])
        m["ct"] = np.ascontiguousarray(context[c * BL:(c + 1) * BL])
        in_maps.append(m)
    return in_maps


def run(in_maps, **kwargs):
    nc = build_program()
    return run_bass_kernel_spmd(nc, in_maps, core_ids=list(range(NCORES)), **kwargs)


def kernel(hidden_states, context, Wq, bq, Wk, bk, Wv, bv):
    in_maps = make_in_maps(hidden_states, context, Wq, bq, Wk, bk, Wv, bv)
    res = run(in_maps)
    outs = [np.asarray(res.results[i]["out"], np.float32) for i in range(NCORES)]
    return np.concatenate(outs, axis=0)
